# revision 4
# baseline (speedup 1.0000x reference)
"""AeroModel (gram-schmidt frame + tiny MLP) Trainium2 kernel.

Self-contained: hardcodes shapes B=2097152, H=32, 8-core data-parallel sharding.
kernel(**inputs) takes full unsharded inputs, returns full [B,3] float32 output.

Math (exactly equivalent to the reference, reformulated to avoid
materializing the rotation matrix):
    nv  = |v|            s1 = 1/nv
    dt  = v.w            f1 = dt*s1            (= w . v_on)
    nw2 = |w|^2 - f1^2   nw = sqrt(nw2)        (= |w_orth|)   s2 = 1/nw
    feat = [nv, f1, nw]
    y = MLP(feat)        (H=32, leaky-relu 0.01, gated 2nd layer)
    out = a*v + b*w + c*(v x w) + bias
      where b = y1*s2, a = s1*(y0 - b*f1), c = y2*s1*s2
"""
import os
import numpy as np
from contextlib import ExitStack

import concourse.bass as bass
import concourse.tile as tile
from concourse import bacc, mybir
from concourse.bass_utils import run_bass_kernel_spmd
from concourse.masks import make_identity

AF = mybir.ActivationFunctionType
ALU = mybir.AluOpType
FP32 = mybir.dt.float32
BF16 = mybir.dt.bfloat16
F32R = mybir.dt.float32r

B = 2097152
NCORES = 8
BC = B // NCORES          # rows per core
P = 128
MG = int(os.environ.get("K_MG", "1024"))  # rows per partition per big-tile
ROWS_BIG = P * MG
G = 4                     # MLP batch groups packed across PE row/col blocks
NSUP = MG // 16           # super-tiles (2048 rows) per big-tile
NMM = 512                 # matmul moving free size
SLOPE = 0.01
LOOP_MODE = os.environ.get("K_LOOP", "plain")
GEO_BUFS = int(os.environ.get("K_GEOBUFS", "1"))
MLP_BUFS = int(os.environ.get("K_MLPBUFS", "4"))
PSM_BUFS = int(os.environ.get("K_PSM", "2"))
PST_BUFS = int(os.environ.get("K_PST", "2"))
PSY_BUFS = int(os.environ.get("K_PSY", "2"))
IL = int(os.environ.get("K_IL", "2"))     # macro interleave width (sw pipeline)


def _body(ctx, tc, nbig, v_d, w_d, wts, out_d, repeat=1, phase='full'):
    nc = tc.nc
    W1_d, b1_d, W2_d, b2_d, Wd1_d, bd1_d, Wd2_d, bd2_d, bias_d = wts

    singles = ctx.enter_context(tc.tile_pool(name="singles", bufs=1))
    geo = ctx.enter_context(tc.tile_pool(name="geo", bufs=GEO_BUFS))
    mlp = ctx.enter_context(tc.tile_pool(name="mlp", bufs=MLP_BUFS))
    psT = ctx.enter_context(tc.tile_pool(name="psT", bufs=PST_BUFS, space="PSUM"))
    psM = ctx.enter_context(tc.tile_pool(name="psM", bufs=PSM_BUFS, space="PSUM"))
    psY = ctx.enter_context(tc.tile_pool(name="psY", bufs=PSY_BUFS, space="PSUM"))

    # ---------------- one-time prep ----------------
    ident = singles.tile([P, P], FP32)
    make_identity(nc, ident)
    identb = singles.tile([P, P], BF16, tag="identb")
    nc.vector.tensor_copy(out=identb, in_=ident)

    # block-diagonal lhsT weights (4 groups); transposes done on PE
    w1q = []   # per-j L1 weights: w1q[j][3(4j+g)+c, 32g+i] = W1[i,c]
    for j in range(4):
        t = singles.tile([48, P], FP32, tag=f"w1q{j}")
        nc.vector.memset(t, 0.0)
        w1q.append(t)
    w2bd = singles.tile([P, P], FP32)        # lhsT[32g+c, 32g+i] = W2[i,c]
    nc.vector.memset(w2bd, 0.0)
    wd1bd = singles.tile([P, P], FP32)
    nc.vector.memset(wd1bd, 0.0)
    wd2bd = singles.tile([P, 12], FP32)      # lhsT[32g+i, 3g+o] = Wd2[o,i]
    nc.vector.memset(wd2bd, 0.0)
    for j in range(4):   # 96 elems each, strided; partition base 3m' so DMA
        for g in range(G):
            mprime = 4 * j + g
            nc.sync.dma_start(
                out=w1q[j][3 * mprime:3 * mprime + 3, 32 * g:32 * g + 32],
                in_=W1_d.rearrange("i c -> c i"))
    for raw_d, shp, bd, blk in ((W2_d, (32, 32), w2bd, 32),
                                (Wd1_d, (32, 32), wd1bd, 32),
                                (Wd2_d, (3, 32), wd2bd, 3)):
        raw = singles.tile(list(shp), FP32, tag=f"raw{id(bd)}")
        nc.sync.dma_start(out=raw, in_=raw_d)
        tps = psY.tile([shp[1], shp[0]], FP32, tag="ytp")
        nc.tensor.transpose(tps, raw, ident[:shp[0], :shp[0]])
        for g in range(G):
            nc.vector.tensor_copy(
                out=bd[32 * g:32 * g + 32, blk * g:blk * g + blk], in_=tps)

    # bf16 copies of the matmul weights (PE runs bf16 at 1 cyc/col vs fp32 4)
    w1qb = []
    for j in range(4):
        t = singles.tile([48, P], BF16, tag=f"w1qb{j}")
        nc.vector.tensor_copy(out=t, in_=w1q[j])
        w1qb.append(t)
    w2bdb = singles.tile([P, P], BF16, tag="w2bdb")
    nc.vector.tensor_copy(out=w2bdb, in_=w2bd)
    wd1bdb = singles.tile([P, P], BF16, tag="wd1bdb")
    nc.vector.tensor_copy(out=wd1bdb, in_=wd1bd)
    wd2bdb = singles.tile([P, 12], BF16, tag="wd2bdb")
    nc.vector.tensor_copy(out=wd2bdb, in_=wd2bd)

    def bcast_vec(src_ap, n, reps, tag):   # DRAM [n] -> SBUF [reps*n, 1] tiled
        t = singles.tile([reps * n, 1], FP32, tag=tag)
        src = bass.AP(tensor=src_ap.tensor, offset=src_ap.offset,
                      ap=[[0, reps], [1, n]])
        nc.sync.dma_start(out=t, in_=src)
        return t

    b1bd = bcast_vec(b1_d, 32, G, "b1bd")     # [128,1]
    b2bd = bcast_vec(b2_d, 32, G, "b2bd")
    bd1bd = bcast_vec(bd1_d, 32, G, "bd1bd")
    bd2b = []        # bd2 component broadcast to all partitions
    for c in range(3):
        t = singles.tile([P, 1], FP32, tag=f"bd2b{c}")
        src_ap = bass.AP(tensor=bd2_d.tensor, offset=bd2_d.offset + c,
                         ap=[[0, P], [1, 1]])
        nc.sync.dma_start(out=t, in_=src_ap)
        bd2b.append(t)
    # world bias components broadcast to all partitions: bias_d is [1,3]
    bzc = []
    for c in range(3):
        t = singles.tile([P, 1], FP32, tag=f"bz{c}")
        src = bass.AP(tensor=bias_d.tensor, offset=bias_d.offset + c,
                      ap=[[0, P], [1, 1]])
        nc.sync.dma_start(out=t, in_=src)
        bzc.append(t)

    # ---------------- main loop (hardware For_i: one body in the NEFF) ----
    # DRAM views with the big-tile index b as an explicit leading dim, so the
    # loop body can slice them with a symbolic loop variable (pure offset).
    v_bt = v_d.rearrange("(b p m) c -> b p (m c)", b=nbig, p=P)
    w_bt = w_d.rearrange("(b p m) c -> b p (m c)", b=nbig, p=P)
    o_bt = out_d.rearrange("(b p m) c -> b p (m c)", b=nbig, p=P)

    def big_tile(bi):
        v3 = geo.tile([P, MG, 3], FP32, tag="v3")
        w3 = geo.tile([P, MG, 3], FP32, tag="w3")
        nc.sync.dma_start(
            out=v3.rearrange("p m c -> p (m c)"),
            in_=v_bt[bass.ds(bi, 1), :, :])
        nc.sync.dma_start(
            out=w3.rearrange("p m c -> p (m c)"),
            in_=w_bt[bass.ds(bi, 1), :, :])

        if phase == "dmaonly":
            nc.sync.dma_start(
                out=o_bt[bass.ds(bi, 1), :, :],
                in_=v3.rearrange("p m c -> p (m c)"))
            return
        FEAT = geo.tile([P, MG, 3], BF16, tag="feat")
        PL = geo.tile([P, 3, MG], FP32, tag="PL")   # [nv, f1, nw] planes
        nv_f = PL[:, 0, :]
        f1f = PL[:, 1, :]
        nw_f = PL[:, 2, :]
        cvw = geo.tile([P, 3, MG], FP32, tag="cvw")
        s1 = geo.tile([P, MG], FP32, tag="s1")
        s2 = geo.tile([P, MG], FP32, tag="s2")
        Y = geo.tile([P, MG, 3], FP32, tag="Y")
        oby = geo.tile([P, MG, 3], FP32, tag="oby")

        sqi = geo.tile([P, MG, 3], FP32, tag="sqi")     # squares/products interleaved
        ta = geo.tile([P, MG], FP32, tag="ta")
        tb = geo.tile([P, MG], FP32, tag="tb")
        tcp = geo.tile([P, MG], FP32, tag="tc")

        # |v|^2 -> nv -> s1
        nc.scalar.activation(out=sqi, in_=v3, func=AF.Square)
        nc.vector.tensor_add(ta, sqi[:, :, 0], sqi[:, :, 1])
        nc.vector.tensor_add(ta, ta, sqi[:, :, 2])
        nc.scalar.activation(out=nv_f, in_=ta, func=AF.Sqrt)
        nc.vector.reciprocal_approx_fast(out=s1, in_=nv_f)


        # v.w -> dt -> f1
        sqj = geo.tile([P, MG, 3], FP32, tag="sqj")
        nc.vector.tensor_mul(sqj, v3, w3)
        nc.vector.tensor_add(tb, sqj[:, :, 0], sqj[:, :, 1])
        nc.vector.tensor_add(tb, tb, sqj[:, :, 2])
        nc.vector.tensor_mul(f1f, tb, s1)               # f1


        # |w|^2 - f1^2 -> nw -> s2
        nc.scalar.activation(out=sqj, in_=w3, func=AF.Square)
        nc.vector.tensor_add(tcp, sqj[:, :, 0], sqj[:, :, 1])
        nc.vector.tensor_add(tcp, tcp, sqj[:, :, 2])
        nc.vector.tensor_mul(ta, f1f, f1f)
        nc.vector.tensor_sub(tcp, tcp, ta)
        nc.scalar.activation(out=nw_f, in_=tcp, func=AF.Sqrt)
        nc.vector.reciprocal_approx_fast(out=s2, in_=nw_f)
        nc.gpsimd.tensor_copy(out=FEAT, in_=PL.rearrange("p c m -> p m c"))

        # cross product v x w (split across gpsimd / dve)
        for (c, i, j) in ((0, 1, 2), (1, 2, 0), (2, 0, 1)):
            tx = geo.tile([P, MG], FP32, tag="tx")
            ty = geo.tile([P, MG], FP32, tag="ty")
            nc.gpsimd.tensor_mul(tx, v3[:, :, i], w3[:, :, j])
            nc.vector.tensor_mul(ty, v3[:, :, j], w3[:, :, i])
            nc.vector.tensor_sub(cvw[:, c, :], tx, ty)

        # ---------------- MLP over macro-tiles (2 super-tiles) ----------------
        # Stage-major emission across IL macros: engine queues are in-order,
        # so issuing macro B's matmuls before macro A's layer-2 lets PE run
        # while ACT/DVE evacuate A's PSUM (software pipelining by reorder).
        if phase == "nomlp":
            nc.gpsimd.memset(Y, 0.0)
        NM2 = 2 * NMM                          # 1024 columns per macro
        nmac = NSUP // 2 if phase != "nomlp" else 0

        def st_l1(mk):
            m0 = 32 * mk
            ftp = psT.tile([48, 2 * P], BF16, tag="ftp")
            for t in range(2):
                gv = FEAT[:, m0 + 16 * t:m0 + 16 * t + 16, :].rearrange(
                    "p m c -> p (m c)")        # [128,48]
                nc.tensor.transpose(ftp[:, P * t:P * (t + 1)], gv, identb)
            rhs1 = mlp.tile([48, 2 * P], BF16, tag="rhs1")
            nc.vector.tensor_copy(out=rhs1, in_=ftp)
            h1ps = psM.tile([P, NM2], FP32, tag="mm")
            for t in range(2):
                for j in range(4):
                    nc.tensor.matmul(
                        h1ps[:, NMM * t + P * j:NMM * t + P * (j + 1)],
                        w1qb[j], rhs1[:, P * t:P * (t + 1)],
                        start=True, stop=True)
            return h1ps

        def st_h1(mk, h1ps):
            h1 = mlp.tile([P, NM2], BF16, tag="h1")
            nc.scalar.activation(out=h1, in_=h1ps, func=AF.Prelu,
                                 bias=b1bd, alpha=SLOPE)
            return h1

        def st_l2(mk, h1):
            ups = psM.tile([P, NM2], FP32, tag="mm")
            for t in range(2):
                nc.tensor.matmul(ups[:, NMM * t:NMM * (t + 1)], w2bdb,
                                 h1[:, NMM * t:NMM * (t + 1)],
                                 start=True, stop=True)
            return ups

        def st_gate(mk, ups, h1):
            # leaky(ups + b2): first part on ACT (1-op Prelu), rest on DVE
            ul = mlp.tile([P, NM2], BF16, tag="ul")
            SA = 640                        # ACT share of the L2 evacuation
            nc.scalar.activation(out=ul[:, :SA], in_=ups[:, :SA],
                                 func=AF.Prelu, bias=b2bd, alpha=SLOPE)
            ut = mlp.tile([P, NM2 - SA], BF16, tag="ut")
            nc.vector.tensor_scalar_add(ut, ups[:, SA:], b2bd)
            nc.vector.scalar_tensor_tensor(out=ul[:, SA:], in0=ut, scalar=SLOPE,
                                           in1=ut, op0=ALU.mult, op1=ALU.max)
            h = mlp.tile([P, NM2], BF16, tag="h")
            nc.vector.tensor_mul(h[:, :NMM], ul[:, :NMM], h1[:, :NMM])
            nc.gpsimd.tensor_mul(h[:, NMM:], ul[:, NMM:], h1[:, NMM:])
            return h

        def st_l3(mk, h):
            y1ps = psM.tile([P, NM2], FP32, tag="mm")
            for t in range(2):
                nc.tensor.matmul(y1ps[:, NMM * t:NMM * (t + 1)], wd1bdb,
                                 h[:, NMM * t:NMM * (t + 1)],
                                 start=True, stop=True)
            return y1ps

        def st_y1(mk, y1ps):
            y1 = mlp.tile([P, NM2], BF16, tag="y1")
            nc.scalar.activation(out=y1, in_=y1ps, func=AF.Prelu,
                                 bias=bd1bd, alpha=SLOPE)
            return y1

        def st_out(mk, y1):
            m0 = 32 * mk
            ytp = psY.tile([P, 96], FP32, tag="ytp")
            for t in range(2):
                for j in range(4):
                    nc.tensor.matmul(
                        ytp[:, 12 * (4 * t + j):12 * (4 * t + j + 1)],
                        y1[:, NMM * t + P * j:NMM * t + P * (j + 1)], wd2bdb,
                        start=True, stop=True)
            yv = Y[:, m0:m0 + 32, :].rearrange("p m c -> p (m c)")
            nc.vector.tensor_copy(out=yv, in_=ytp)

        for mk0 in range(0, nmac, IL):
            mks = [mk0 + d for d in range(IL) if mk0 + d < nmac]
            st = {}
            for mk in mks:
                st[mk] = [st_l1(mk)]
            for mk in mks:
                st[mk].append(st_h1(mk, st[mk][-1]))
            for mk in mks:
                st[mk].append(st_l2(mk, st[mk][-1]))
            for mk in mks:
                st[mk].append(st_gate(mk, st[mk][-1], st[mk][-2]))
            for mk in mks:
                st[mk].append(st_l3(mk, st[mk][-1]))
            for mk in mks:
                st[mk].append(st_y1(mk, st[mk][-1]))
            for mk in mks:
                st_out(mk, st[mk][-1])

        # ---------------- back-end rotation ----------------
        bsc = geo.tile([P, MG], FP32, tag="bsc")
        asc = geo.tile([P, MG], FP32, tag="asc")
        csc = geo.tile([P, MG], FP32, tag="csc")
        # b = (y1 + bd2_1) * s2
        nc.vector.scalar_tensor_tensor(out=bsc, in0=Y[:, :, 1], scalar=bd2b[1],
                                       in1=s2, op0=ALU.add, op1=ALU.mult)
        nc.vector.tensor_mul(ta, bsc, f1f)                      # b*f1
        # tb = (y0 + bd2_0) - b*f1
        nc.vector.scalar_tensor_tensor(out=tb, in0=Y[:, :, 0], scalar=bd2b[0],
                                       in1=ta, op0=ALU.add, op1=ALU.subtract)
        nc.vector.tensor_mul(asc, s1, tb)                       # a
        # y2*s1 with bd2_2 folded
        nc.vector.scalar_tensor_tensor(out=tcp, in0=Y[:, :, 2], scalar=bd2b[2],
                                       in1=s1, op0=ALU.add, op1=ALU.mult)
        nc.vector.tensor_mul(csc, tcp, s2)                      # c

        for c in range(3):
            o1 = geo.tile([P, MG], FP32, tag="o1")
            t3 = geo.tile([P, MG], FP32, tag="t3")
            t4 = geo.tile([P, MG], FP32, tag="t4")
            nc.gpsimd.tensor_mul(o1, asc, v3[:, :, c])
            nc.vector.tensor_mul(t3, bsc, w3[:, :, c])
            nc.gpsimd.tensor_add(o1, o1, t3)
            nc.vector.tensor_mul(t4, csc, cvw[:, c, :])
            # out = (o1 + bias_c) + t4  in one fused DVE op
            nc.vector.scalar_tensor_tensor(out=oby[:, :, c], in0=o1,
                                           scalar=bzc[c], in1=t4,
                                           op0=ALU.add, op1=ALU.add)

        nc.scalar.dma_start(
            out=o_bt[bass.ds(bi, 1), :, :],
            in_=oby.rearrange("p m c -> p (m c)"))

    if LOOP_MODE == "python":          # fully unrolled (for TimelineSim)
        for _ in range(repeat):
            for b in range(nbig):
                big_tile(b)
    else:
        with tc.For_i(0, repeat, 1):
            with tc.For_i(0, nbig, 1) as bi:
                big_tile(bi)


def _build(nbig, repeat=1, phase='full'):
    nc = bacc.Bacc("TRN2", target_bir_lowering=False, debug=False,
                   num_devices=NCORES)
    rows = nbig * ROWS_BIG
    v_d = nc.dram_tensor("v", [rows, 3], FP32, kind="ExternalInput").ap()
    w_d = nc.dram_tensor("w", [rows, 3], FP32, kind="ExternalInput").ap()
    W1_d = nc.dram_tensor("W1", [32, 3], FP32, kind="ExternalInput").ap()
    b1_d = nc.dram_tensor("b1", [32], FP32, kind="ExternalInput").ap()
    W2_d = nc.dram_tensor("W2", [32, 32], FP32, kind="ExternalInput").ap()
    b2_d = nc.dram_tensor("b2", [32], FP32, kind="ExternalInput").ap()
    Wd1_d = nc.dram_tensor("Wd1", [32, 32], FP32, kind="ExternalInput").ap()
    bd1_d = nc.dram_tensor("bd1", [32], FP32, kind="ExternalInput").ap()
    Wd2_d = nc.dram_tensor("Wd2", [3, 32], FP32, kind="ExternalInput").ap()
    bd2_d = nc.dram_tensor("bd2", [3], FP32, kind="ExternalInput").ap()
    bias_d = nc.dram_tensor("bias", [1, 3], FP32, kind="ExternalInput").ap()
    out_d = nc.dram_tensor("out", [rows, 3], FP32, kind="ExternalOutput").ap()

    wts = (W1_d, b1_d, W2_d, b2_d, Wd1_d, bd1_d, Wd2_d, bd2_d, bias_d)
    with tile.TileContext(nc) as tc:
        with ExitStack() as ctx:
            _body(ctx, tc, nbig, v_d, w_d, wts, out_d, repeat, phase)
    nc.compile()
    return nc


_NC_CACHE = {}


def _get_nc(nbig, repeat=1, phase="full"):
    key = (nbig, repeat, phase, LOOP_MODE, PSM_BUFS, PST_BUFS, PSY_BUFS, IL)
    if key not in _NC_CACHE:
        _NC_CACHE[key] = _build(nbig, repeat, phase)
    return _NC_CACHE[key]


WNAMES = ["W1", "b1", "W2", "b2", "Wd1", "bd1", "Wd2", "bd2", "bias"]


def _run(v, w, wdict, nbig, n_cores, trace=False, repeat=1, phase="full"):
    nc = _get_nc(nbig, repeat, phase)
    rows = nbig * ROWS_BIG
    in_maps = []
    for c in range(n_cores):
        m = {"v": np.ascontiguousarray(v[c * rows:(c + 1) * rows]),
             "w": np.ascontiguousarray(w[c * rows:(c + 1) * rows])}
        for k in WNAMES:
            m[k] = wdict[k]
        in_maps.append(m)
    last_err = None
    for attempt in range(3):
        try:
            res = run_bass_kernel_spmd(nc, in_maps,
                                       core_ids=list(range(n_cores)),
                                       trace=trace)
            break
        except Exception as e:      # transient NRT device errors
            last_err = e
            import time as _t
            _t.sleep(5)
    else:
        raise last_err
    out = np.concatenate([res.results[c]["out"] for c in range(n_cores)], axis=0)
    return out, res


def kernel(**inputs):
    v = np.ascontiguousarray(np.asarray(inputs["v"], dtype=np.float32))
    w = np.ascontiguousarray(np.asarray(inputs["w"], dtype=np.float32))
    wdict = {k: np.ascontiguousarray(np.asarray(inputs[k], dtype=np.float32))
             for k in WNAMES}
    wdict["bias"] = wdict["bias"].reshape(1, 3)
    out, _ = _run(v, w, wdict, BC // ROWS_BIG, NCORES)
    return out



# revision 5
# speedup vs baseline: 1.4845x; 1.4845x over previous
"""AeroModel (gram-schmidt frame + tiny MLP) Trainium2 kernel.

Self-contained: hardcodes shapes B=2097152, H=32, 8-core data-parallel sharding.
kernel(**inputs) takes full unsharded inputs, returns full [B,3] float32 output.

Math (exactly equivalent to the reference, reformulated to avoid
materializing the rotation matrix):
    nv  = |v|            s1 = 1/nv
    dt  = v.w            f1 = dt*s1            (= w . v_on)
    nw2 = |w|^2 - f1^2   nw = sqrt(nw2)        (= |w_orth|)   s2 = 1/nw
    feat = [nv, f1, nw]
    y = MLP(feat)        (H=32, leaky-relu 0.01, gated 2nd layer)
    out = a*v + b*w + c*(v x w) + bias
      where b = y1*s2, a = s1*(y0 - b*f1), c = y2*s1*s2
"""
import os
import numpy as np
from contextlib import ExitStack

import concourse.bass as bass
import concourse.tile as tile
from concourse import bacc, mybir
from concourse.bass_utils import run_bass_kernel_spmd
from concourse.masks import make_identity

AF = mybir.ActivationFunctionType
ALU = mybir.AluOpType
FP32 = mybir.dt.float32
BF16 = mybir.dt.bfloat16
F32R = mybir.dt.float32r

B = 2097152
NCORES = 8
BC = B // NCORES          # rows per core
P = 128
MG = int(os.environ.get("K_MG", "1024"))  # rows per partition per big-tile
ROWS_BIG = P * MG
G = 4                     # MLP batch groups packed across PE row/col blocks
NSUP = MG // 16           # super-tiles (2048 rows) per big-tile
NMM = 512                 # matmul moving free size
SLOPE = 0.01
LOOP_MODE = os.environ.get("K_LOOP", "plain")
GEO_BUFS = int(os.environ.get("K_GEOBUFS", "1"))
MLP_BUFS = int(os.environ.get("K_MLPBUFS", "4"))
PSM_BUFS = int(os.environ.get("K_PSM", "2"))
PST_BUFS = int(os.environ.get("K_PST", "2"))
PSY_BUFS = int(os.environ.get("K_PSY", "2"))
IL = int(os.environ.get("K_IL", "2"))     # macro interleave width (sw pipeline)


def _body(ctx, tc, nbig, v_d, w_d, wts, out_d, repeat=1, phase='full'):
    nc = tc.nc
    W1_d, b1_d, W2_d, b2_d, Wd1_d, bd1_d, Wd2_d, bd2_d, bias_d = wts

    singles = ctx.enter_context(tc.tile_pool(name="singles", bufs=1))
    geo = ctx.enter_context(tc.tile_pool(name="geo", bufs=GEO_BUFS))
    mlp = ctx.enter_context(tc.tile_pool(name="mlp", bufs=MLP_BUFS))
    psT = ctx.enter_context(tc.tile_pool(name="psT", bufs=PST_BUFS, space="PSUM"))
    psM = ctx.enter_context(tc.tile_pool(name="psM", bufs=PSM_BUFS, space="PSUM"))
    psY = ctx.enter_context(tc.tile_pool(name="psY", bufs=PSY_BUFS, space="PSUM"))

    # ---------------- one-time prep ----------------
    ident = singles.tile([P, P], FP32)
    make_identity(nc, ident)
    identb = singles.tile([P, P], BF16, tag="identb")
    nc.vector.tensor_copy(out=identb, in_=ident)

    # block-diagonal lhsT weights (4 groups); transposes done on PE
    # 8 jj-blocks over 32-row macro transposes: w1q[jj][3(4jj+g)+c, 32g+i]
    w1q = []
    for jj in range(8):
        t = singles.tile([96, P], FP32, tag=f"w1q{jj}")
        nc.vector.memset(t, 0.0)
        w1q.append(t)
    w2bd = singles.tile([P, P], FP32)        # lhsT[32g+c, 32g+i] = W2[i,c]
    nc.vector.memset(w2bd, 0.0)
    wd1bd = singles.tile([P, P], FP32)
    nc.vector.memset(wd1bd, 0.0)
    wd2bd = singles.tile([P, 12], FP32)      # lhsT[32g+i, 3g+o] = Wd2[o,i]
    nc.vector.memset(wd2bd, 0.0)
    for jj in range(8):  # 96 elems each, strided; partition base 3m' so DMA
        for g in range(G):
            mprime = 4 * jj + g
            nc.sync.dma_start(
                out=w1q[jj][3 * mprime:3 * mprime + 3, 32 * g:32 * g + 32],
                in_=W1_d.rearrange("i c -> c i"))
    for raw_d, shp, bd, blk in ((W2_d, (32, 32), w2bd, 32),
                                (Wd1_d, (32, 32), wd1bd, 32),
                                (Wd2_d, (3, 32), wd2bd, 3)):
        raw = singles.tile(list(shp), FP32, tag=f"raw{id(bd)}")
        nc.sync.dma_start(out=raw, in_=raw_d)
        tps = psY.tile([shp[1], shp[0]], FP32, tag="ytp")
        nc.tensor.transpose(tps, raw, ident[:shp[0], :shp[0]])
        for g in range(G):
            nc.vector.tensor_copy(
                out=bd[32 * g:32 * g + 32, blk * g:blk * g + blk], in_=tps)

    # bf16 copies of the matmul weights (PE runs bf16 at 1 cyc/col vs fp32 4)
    w1qb = []
    for jj in range(8):
        t = singles.tile([96, P], BF16, tag=f"w1qb{jj}")
        nc.vector.tensor_copy(out=t, in_=w1q[jj])
        w1qb.append(t)
    w2bdb = singles.tile([P, P], BF16, tag="w2bdb")
    nc.vector.tensor_copy(out=w2bdb, in_=w2bd)
    wd1bdb = singles.tile([P, P], BF16, tag="wd1bdb")
    nc.vector.tensor_copy(out=wd1bdb, in_=wd1bd)
    wd2bdb = singles.tile([P, 12], BF16, tag="wd2bdb")
    nc.vector.tensor_copy(out=wd2bdb, in_=wd2bd)

    def bcast_vec(src_ap, n, reps, tag):   # DRAM [n] -> SBUF [reps*n, 1] tiled
        t = singles.tile([reps * n, 1], FP32, tag=tag)
        src = bass.AP(tensor=src_ap.tensor, offset=src_ap.offset,
                      ap=[[0, reps], [1, n]])
        nc.sync.dma_start(out=t, in_=src)
        return t

    b1bd = bcast_vec(b1_d, 32, G, "b1bd")     # [128,1]
    b2bd = bcast_vec(b2_d, 32, G, "b2bd")
    bd1bd = bcast_vec(bd1_d, 32, G, "bd1bd")
    bd2b = []        # bd2 component broadcast to all partitions
    for c in range(3):
        t = singles.tile([P, 1], FP32, tag=f"bd2b{c}")
        src_ap = bass.AP(tensor=bd2_d.tensor, offset=bd2_d.offset + c,
                         ap=[[0, P], [1, 1]])
        nc.sync.dma_start(out=t, in_=src_ap)
        bd2b.append(t)
    # world bias components broadcast to all partitions: bias_d is [1,3]
    bzc = []
    for c in range(3):
        t = singles.tile([P, 1], FP32, tag=f"bz{c}")
        src = bass.AP(tensor=bias_d.tensor, offset=bias_d.offset + c,
                      ap=[[0, P], [1, 1]])
        nc.sync.dma_start(out=t, in_=src)
        bzc.append(t)

    # ---------------- main loop (hardware For_i: one body in the NEFF) ----
    # DRAM views with the big-tile index b as an explicit leading dim, so the
    # loop body can slice them with a symbolic loop variable (pure offset).
    v_bt = v_d.rearrange("(b p m) c -> b p (m c)", b=nbig, p=P)
    w_bt = w_d.rearrange("(b p m) c -> b p (m c)", b=nbig, p=P)
    o_bt = out_d.rearrange("(b p m) c -> b p (m c)", b=nbig, p=P)

    def big_tile(bi):
        v3 = geo.tile([P, MG, 3], FP32, tag="v3")
        w3 = geo.tile([P, MG, 3], FP32, tag="w3")
        nc.sync.dma_start(
            out=v3.rearrange("p m c -> p (m c)"),
            in_=v_bt[bass.ds(bi, 1), :, :])
        nc.sync.dma_start(
            out=w3.rearrange("p m c -> p (m c)"),
            in_=w_bt[bass.ds(bi, 1), :, :])

        if phase == "dmaonly":
            nc.sync.dma_start(
                out=o_bt[bass.ds(bi, 1), :, :],
                in_=v3.rearrange("p m c -> p (m c)"))
            return
        FEAT = geo.tile([P, MG, 3], BF16, tag="feat")
        PL = geo.tile([P, 3, MG], FP32, tag="PL")   # [nv, f1, nw] planes
        nv_f = PL[:, 0, :]
        f1f = PL[:, 1, :]
        nw_f = PL[:, 2, :]
        cvw = geo.tile([P, 3, MG], FP32, tag="cvw")
        s1 = geo.tile([P, MG], FP32, tag="s1")
        s2 = geo.tile([P, MG], FP32, tag="s2")
        Y = geo.tile([P, MG, 3], FP32, tag="Y")
        oby = geo.tile([P, MG, 3], FP32, tag="oby")

        sqi = geo.tile([P, MG, 3], FP32, tag="sqi")     # squares/products interleaved
        ta = geo.tile([P, MG], FP32, tag="ta")
        tb = geo.tile([P, MG], FP32, tag="tb")
        tcp = geo.tile([P, MG], FP32, tag="tc")

        # |v|^2 -> nv -> s1
        nc.scalar.activation(out=sqi, in_=v3, func=AF.Square)
        nc.vector.tensor_add(ta, sqi[:, :, 0], sqi[:, :, 1])
        nc.vector.tensor_add(ta, ta, sqi[:, :, 2])
        nc.scalar.activation(out=nv_f, in_=ta, func=AF.Sqrt)
        nc.vector.reciprocal_approx_fast(out=s1, in_=nv_f)


        # v.w -> dt -> f1
        sqj = geo.tile([P, MG, 3], FP32, tag="sqj")
        nc.vector.tensor_mul(sqj, v3, w3)
        nc.vector.tensor_add(tb, sqj[:, :, 0], sqj[:, :, 1])
        nc.vector.tensor_add(tb, tb, sqj[:, :, 2])
        nc.vector.tensor_mul(f1f, tb, s1)               # f1


        # |w|^2 - f1^2 -> nw -> s2
        nc.scalar.activation(out=sqj, in_=w3, func=AF.Square)
        nc.vector.tensor_add(tcp, sqj[:, :, 0], sqj[:, :, 1])
        nc.vector.tensor_add(tcp, tcp, sqj[:, :, 2])
        nc.vector.tensor_mul(ta, f1f, f1f)
        nc.vector.tensor_sub(tcp, tcp, ta)
        nc.scalar.activation(out=nw_f, in_=tcp, func=AF.Sqrt)
        nc.vector.reciprocal_approx_fast(out=s2, in_=nw_f)
        nc.gpsimd.tensor_copy(out=FEAT, in_=PL.rearrange("p c m -> p m c"))

        # cross product v x w (split across gpsimd / dve)
        for (c, i, j) in ((0, 1, 2), (1, 2, 0), (2, 0, 1)):
            tx = geo.tile([P, MG], FP32, tag="tx")
            ty = geo.tile([P, MG], FP32, tag="ty")
            nc.gpsimd.tensor_mul(tx, v3[:, :, i], w3[:, :, j])
            nc.vector.tensor_mul(ty, v3[:, :, j], w3[:, :, i])
            nc.vector.tensor_sub(cvw[:, c, :], tx, ty)

        # ---------------- MLP over macro-tiles (2 super-tiles) ----------------
        # Stage-major emission across IL macros: engine queues are in-order,
        # so issuing macro B's matmuls before macro A's layer-2 lets PE run
        # while ACT/DVE evacuate A's PSUM (software pipelining by reorder).
        if phase == "nomlp":
            nc.gpsimd.memset(Y, 0.0)
        NM2 = 2 * NMM                          # 1024 columns per macro
        nmac = NSUP // 2 if phase != "nomlp" else 0

        def st_l1(mk):
            # one 32-row transpose [128,96] -> [96,128]; L1 as 8 jj-block
            # matmuls sharing the single rhs (block jj covers rows 4jj+g)
            m0 = 32 * mk
            gv = FEAT[:, m0:m0 + 32, :].rearrange("p m c -> p (m c)")  # [128,96]
            ftp = psT.tile([96, P], BF16, tag="ftp")
            nc.tensor.transpose(ftp, gv, identb)
            rhs1 = mlp.tile([96, P], BF16, tag="rhs1")
            nc.vector.tensor_copy(out=rhs1, in_=ftp)
            h1ps = psM.tile([P, NM2], FP32, tag="mm")
            for jj in range(8):
                nc.tensor.matmul(
                    h1ps[:, P * jj:P * (jj + 1)],
                    w1qb[jj], rhs1,
                    start=True, stop=True)
            return h1ps

        def st_h1(mk, h1ps):
            h1 = mlp.tile([P, NM2], BF16, tag="h1")
            nc.scalar.activation(out=h1, in_=h1ps, func=AF.Prelu,
                                 bias=b1bd, alpha=SLOPE)
            return h1

        def st_l2(mk, h1):
            ups = psM.tile([P, NM2], FP32, tag="mm")
            for t in range(2):
                nc.tensor.matmul(ups[:, NMM * t:NMM * (t + 1)], w2bdb,
                                 h1[:, NMM * t:NMM * (t + 1)],
                                 start=True, stop=True)
            return ups

        def st_gate(mk, ups, h1):
            # leaky(ups + b2): first part on ACT (1-op Prelu), rest on DVE
            ul = mlp.tile([P, NM2], BF16, tag="ul")
            SA = 640                        # ACT share of the L2 evacuation
            nc.scalar.activation(out=ul[:, :SA], in_=ups[:, :SA],
                                 func=AF.Prelu, bias=b2bd, alpha=SLOPE)
            ut = mlp.tile([P, NM2 - SA], BF16, tag="ut")
            nc.vector.tensor_scalar_add(ut, ups[:, SA:], b2bd)
            nc.vector.scalar_tensor_tensor(out=ul[:, SA:], in0=ut, scalar=SLOPE,
                                           in1=ut, op0=ALU.mult, op1=ALU.max)
            h = mlp.tile([P, NM2], BF16, tag="h")
            nc.vector.tensor_mul(h[:, :NMM], ul[:, :NMM], h1[:, :NMM])
            nc.gpsimd.tensor_mul(h[:, NMM:], ul[:, NMM:], h1[:, NMM:])
            return h

        def st_l3(mk, h):
            y1ps = psM.tile([P, NM2], FP32, tag="mm")
            for t in range(2):
                nc.tensor.matmul(y1ps[:, NMM * t:NMM * (t + 1)], wd1bdb,
                                 h[:, NMM * t:NMM * (t + 1)],
                                 start=True, stop=True)
            return y1ps

        def st_y1(mk, y1ps):
            y1 = mlp.tile([P, NM2], BF16, tag="y1")
            nc.scalar.activation(out=y1, in_=y1ps, func=AF.Prelu,
                                 bias=bd1bd, alpha=SLOPE)
            return y1

        def st_out(mk, y1):
            m0 = 32 * mk
            ytp = psY.tile([P, 96], FP32, tag="ytp")
            for jj in range(8):
                nc.tensor.matmul(
                    ytp[:, 12 * jj:12 * (jj + 1)],
                    y1[:, P * jj:P * (jj + 1)], wd2bdb,
                    start=True, stop=True)
            yv = Y[:, m0:m0 + 32, :].rearrange("p m c -> p (m c)")
            nc.vector.tensor_copy(out=yv, in_=ytp)

        for mk0 in range(0, nmac, IL):
            mks = [mk0 + d for d in range(IL) if mk0 + d < nmac]
            st = {}
            for mk in mks:
                st[mk] = [st_l1(mk)]
            for mk in mks:
                st[mk].append(st_h1(mk, st[mk][-1]))
            for mk in mks:
                st[mk].append(st_l2(mk, st[mk][-1]))
            for mk in mks:
                st[mk].append(st_gate(mk, st[mk][-1], st[mk][-2]))
            for mk in mks:
                st[mk].append(st_l3(mk, st[mk][-1]))
            for mk in mks:
                st[mk].append(st_y1(mk, st[mk][-1]))
            for mk in mks:
                st_out(mk, st[mk][-1])

        # ---------------- back-end rotation ----------------
        bsc = geo.tile([P, MG], FP32, tag="bsc")
        asc = geo.tile([P, MG], FP32, tag="asc")
        csc = geo.tile([P, MG], FP32, tag="csc")
        # b = (y1 + bd2_1) * s2
        nc.vector.scalar_tensor_tensor(out=bsc, in0=Y[:, :, 1], scalar=bd2b[1],
                                       in1=s2, op0=ALU.add, op1=ALU.mult)
        nc.vector.tensor_mul(ta, bsc, f1f)                      # b*f1
        # tb = (y0 + bd2_0) - b*f1
        nc.vector.scalar_tensor_tensor(out=tb, in0=Y[:, :, 0], scalar=bd2b[0],
                                       in1=ta, op0=ALU.add, op1=ALU.subtract)
        nc.vector.tensor_mul(asc, s1, tb)                       # a
        # y2*s1 with bd2_2 folded
        nc.vector.scalar_tensor_tensor(out=tcp, in0=Y[:, :, 2], scalar=bd2b[2],
                                       in1=s1, op0=ALU.add, op1=ALU.mult)
        nc.vector.tensor_mul(csc, tcp, s2)                      # c

        for c in range(3):
            o1 = geo.tile([P, MG], FP32, tag="o1")
            t3 = geo.tile([P, MG], FP32, tag="t3")
            t4 = geo.tile([P, MG], FP32, tag="t4")
            nc.gpsimd.tensor_mul(o1, asc, v3[:, :, c])
            nc.vector.tensor_mul(t3, bsc, w3[:, :, c])
            nc.gpsimd.tensor_add(o1, o1, t3)
            nc.vector.tensor_mul(t4, csc, cvw[:, c, :])
            # out = (o1 + bias_c) + t4  in one fused DVE op
            nc.vector.scalar_tensor_tensor(out=oby[:, :, c], in0=o1,
                                           scalar=bzc[c], in1=t4,
                                           op0=ALU.add, op1=ALU.add)

        nc.scalar.dma_start(
            out=o_bt[bass.ds(bi, 1), :, :],
            in_=oby.rearrange("p m c -> p (m c)"))

    if LOOP_MODE == "python":          # fully unrolled (for TimelineSim)
        for _ in range(repeat):
            for b in range(nbig):
                big_tile(b)
    else:
        with tc.For_i(0, repeat, 1):
            with tc.For_i(0, nbig, 1) as bi:
                big_tile(bi)


def _build(nbig, repeat=1, phase='full'):
    nc = bacc.Bacc("TRN2", target_bir_lowering=False, debug=False,
                   num_devices=NCORES)
    rows = nbig * ROWS_BIG
    v_d = nc.dram_tensor("v", [rows, 3], FP32, kind="ExternalInput").ap()
    w_d = nc.dram_tensor("w", [rows, 3], FP32, kind="ExternalInput").ap()
    W1_d = nc.dram_tensor("W1", [32, 3], FP32, kind="ExternalInput").ap()
    b1_d = nc.dram_tensor("b1", [32], FP32, kind="ExternalInput").ap()
    W2_d = nc.dram_tensor("W2", [32, 32], FP32, kind="ExternalInput").ap()
    b2_d = nc.dram_tensor("b2", [32], FP32, kind="ExternalInput").ap()
    Wd1_d = nc.dram_tensor("Wd1", [32, 32], FP32, kind="ExternalInput").ap()
    bd1_d = nc.dram_tensor("bd1", [32], FP32, kind="ExternalInput").ap()
    Wd2_d = nc.dram_tensor("Wd2", [3, 32], FP32, kind="ExternalInput").ap()
    bd2_d = nc.dram_tensor("bd2", [3], FP32, kind="ExternalInput").ap()
    bias_d = nc.dram_tensor("bias", [1, 3], FP32, kind="ExternalInput").ap()
    out_d = nc.dram_tensor("out", [rows, 3], FP32, kind="ExternalOutput").ap()

    wts = (W1_d, b1_d, W2_d, b2_d, Wd1_d, bd1_d, Wd2_d, bd2_d, bias_d)
    with tile.TileContext(nc) as tc:
        with ExitStack() as ctx:
            _body(ctx, tc, nbig, v_d, w_d, wts, out_d, repeat, phase)
    nc.compile()
    return nc


_NC_CACHE = {}


def _get_nc(nbig, repeat=1, phase="full"):
    key = (nbig, repeat, phase, LOOP_MODE, PSM_BUFS, PST_BUFS, PSY_BUFS, IL)
    if key not in _NC_CACHE:
        _NC_CACHE[key] = _build(nbig, repeat, phase)
    return _NC_CACHE[key]


WNAMES = ["W1", "b1", "W2", "b2", "Wd1", "bd1", "Wd2", "bd2", "bias"]


def _run(v, w, wdict, nbig, n_cores, trace=False, repeat=1, phase="full"):
    nc = _get_nc(nbig, repeat, phase)
    rows = nbig * ROWS_BIG
    in_maps = []
    for c in range(n_cores):
        m = {"v": np.ascontiguousarray(v[c * rows:(c + 1) * rows]),
             "w": np.ascontiguousarray(w[c * rows:(c + 1) * rows])}
        for k in WNAMES:
            m[k] = wdict[k]
        in_maps.append(m)
    last_err = None
    for attempt in range(3):
        try:
            res = run_bass_kernel_spmd(nc, in_maps,
                                       core_ids=list(range(n_cores)),
                                       trace=trace)
            break
        except Exception as e:      # transient NRT device errors
            last_err = e
            import time as _t
            _t.sleep(5)
    else:
        raise last_err
    out = np.concatenate([res.results[c]["out"] for c in range(n_cores)], axis=0)
    return out, res


def kernel(**inputs):
    v = np.ascontiguousarray(np.asarray(inputs["v"], dtype=np.float32))
    w = np.ascontiguousarray(np.asarray(inputs["w"], dtype=np.float32))
    wdict = {k: np.ascontiguousarray(np.asarray(inputs[k], dtype=np.float32))
             for k in WNAMES}
    wdict["bias"] = wdict["bias"].reshape(1, 3)
    out, _ = _run(v, w, wdict, BC // ROWS_BIG, NCORES)
    return out

